# revision 26
# baseline (speedup 1.0000x reference)
"""GQA kernel for Trainium2, sharded over 8 NeuronCores.

Sharding: tensor-parallel over heads. Core g owns Q heads 4g..4g+3 and KV
group g (GQA rep=4, so all 4 local heads share one K/V). The reference's
final projection contracts over the *sequence* axis (faithful swapaxes
quirk), so output rows partition cleanly by head: core g produces rows
g*256..(g+1)*256 of the [2, 2048, 2048] output. No collectives.

v2 schedule (software-pipelined, trace-driven):
  - xt is loaded per-hidden-chunk; projection matmuls start as chunks land.
  - attention inner loop is ACT(exp)-bound; batch-1 projection and batch-0
    out-projection matmul chains are injected between attention matmuls as
    "filler" thunks so the PE never idles while ACT chews exp tiles.
  - all transposes (V, attn, softmax denominator) run on the DMA XBAR
    (dma_start_transpose), not the PE.
  - out-projection is split per head-pair so rows for heads 0/1 of batch 1
    are projected while heads 2/3 still run attention; only the last
    quarter of the out-projection remains after the attention pipeline.
"""
import numpy as np
import ml_dtypes
from collections import deque

import concourse.bass as bass
import concourse.bacc as bacc
import concourse.mybir as mybir
import concourse.tile as tile
from concourse import bass_utils

BF16 = mybir.dt.bfloat16
F32 = mybir.dt.float32
NP_BF16 = ml_dtypes.bfloat16

B, S, HID = 2, 2048, 2048
NCORES = 8
HEADS_PER_CORE = 4   # of 32
D = 64               # head dim
QF = HEADS_PER_CORE * D   # 256 q-features per core
P = 128
HC = HID // P        # 16 hidden chunks
SC = S // P          # 16 seq chunks

_CACHE = {}


def _build():
    nc = bacc.Bacc("TRN2", target_bir_lowering=False, debug=False,
                   num_devices=NCORES)
    # ---- DRAM I/O ----
    xt_d = nc.dram_tensor("xt", [B, HID, S], BF16, kind="ExternalInput").ap()
    # weights arrive pre-arranged on host so every DMA line is contiguous
    wq_d = nc.dram_tensor("wq", [P, HC, QF], BF16, kind="ExternalInput").ap()
    bq_d = nc.dram_tensor("bq", [2, P], F32, kind="ExternalInput").ap()
    wkv_d = nc.dram_tensor("wkv", [P, HC, P], BF16, kind="ExternalInput").ap()
    bkv_d = nc.dram_tensor("bkv", [P, 1], F32, kind="ExternalInput").ap()
    wo_d = nc.dram_tensor("wo", [P, 4, SC, 512], BF16,
                          kind="ExternalInput").ap()
    bo_d = nc.dram_tensor("bo", [P, HID], F32, kind="ExternalInput").ap()
    out_d = nc.dram_tensor("out", [B, QF, HID], F32, kind="ExternalOutput").ap()

    with tile.TileContext(nc) as tc:
        with (
            tc.tile_pool(name="consts", bufs=1) as consts,
            tc.tile_pool(name="xt", bufs=1) as xt_pool,
            tc.tile_pool(name="qt", bufs=2) as qt_pool,
            tc.tile_pool(name="kvt", bufs=2) as kvt_pool,
            tc.tile_pool(name="vp", bufs=2) as vp_pool,
            tc.tile_pool(name="pt", bufs=2) as pt_pool,
            tc.tile_pool(name="attnT", bufs=2) as attnT_pool,
            tc.tile_pool(name="araw", bufs=2) as araw_pool,
            tc.tile_pool(name="dxt", bufs=2) as dxt_pool,
            tc.tile_pool(name="rcp", bufs=2) as rcp_pool,
            tc.tile_pool(name="attn", bufs=2) as attn_pool,
            tc.tile_pool(name="wos", bufs=2) as wos_pool,
            tc.tile_pool(name="outp", bufs=2) as out_pool,
            tc.tile_pool(name="psum", bufs=1, space="PSUM") as psum,
        ):
            # ---- constants (bo deferred: not needed until out-proj) ----
            bq_sb = consts.tile([P, 2], F32)
            nc.sync.dma_start(bq_sb, bq_d.rearrange("c p -> p c"))
            bkv_sb = consts.tile([P, 1], F32)
            nc.sync.dma_start(bkv_sb, bkv_d)
            wq_sb = consts.tile([P, HC, QF], BF16)
            nc.sync.dma_start(wq_sb, wq_d)
            wkv_sb = consts.tile([P, HC, P], BF16)
            nc.sync.dma_start(wkv_sb, wkv_d)
            wqc = [wq_sb[:, hc] for hc in range(HC)]
            wkvc = [wkv_sb[:, hc] for hc in range(HC)]
            bo_sb = consts.tile([P, HID], F32)

            # filler thunk queue: each thunk emits a small chunk of deferred
            # PE work (or a DMA, cost 0) and returns its PE cost in matmuls.
            work = deque()

            def pump(budget=1):
                spent = 0
                while work and spent < budget:
                    spent += work.popleft()()

            def flush():
                while work:
                    work.popleft()()

            # ---------------- projections ----------------
            def load_xt(b):
                """xt in chunk-pairs: fewer triggers, 8KB DMA lines."""
                pairs = []
                for hp in range(HC // 2):
                    t = xt_pool.tile([P, 2, S], BF16, tag=f"xt{hp}",
                                     name=f"xtc{hp}")
                    nc.sync.dma_start(
                        t, xt_d[b, hp * 2 * P:(hp + 1) * 2 * P, :]
                        .rearrange("(c p) s -> p c s", p=P))
                    pairs.append(t)
                return [pairs[hc // 2][:, hc % 2, :] for hc in range(HC)]

            def proj_drain_qt(qt_sb, ps, qc, sh):
                nc.vector.tensor_tensor(
                    out=qt_sb[:, qc, sh * 1024:(sh + 1) * 1024],
                    in0=ps, in1=bq_sb[:, qc:qc + 1].to_broadcast((P, 1024)),
                    op=mybir.AluOpType.add)

            def proj_drain_kv(kvt_sb, ps, sh, j):
                nc.vector.tensor_tensor(
                    out=kvt_sb[:, sh * 1024 + j * 512:sh * 1024 + (j + 1) * 512],
                    in0=ps, in1=bkv_sb[:, 0:1].to_broadcast((P, 512)),
                    op=mybir.AluOpType.add)

            def proj_phase_b0(xt_t, qt_sb, kvt_sb):
                """Paced projection for batch 0: matmuls chase the xt DMAs.
                Round 1 (DMA-paced): qc0 both halves, qc1 sh0, kv sh0.
                Round 2 (chunks resident): qc1 sh1, kv sh1."""
                a0 = psum.tile([P, 1024], F32, tag="A", bufs=2)
                a1 = psum.tile([P, 1024], F32, tag="A", bufs=2)
                a2 = psum.tile([P, 1024], F32, tag="av", bufs=1)
                c0 = psum.tile([P, 512], F32, tag="C", bufs=2)
                c1 = psum.tile([P, 512], F32, tag="C", bufs=2)
                for hc in range(HC):
                    st, sp = hc == 0, hc == HC - 1
                    for qc, sh, ps in ((0, 0, a0), (0, 1, a1), (1, 0, a2)):
                        for j in range(2):
                            nc.tensor.matmul(
                                ps[:, j * 512:(j + 1) * 512],
                                lhsT=wqc[hc][:, qc * P:(qc + 1) * P],
                                rhs=xt_t[hc][:, sh * 1024 + j * 512:
                                             sh * 1024 + (j + 1) * 512],
                                start=st, stop=sp, skip_group_check=True)
                    for j, ps in ((0, c0), (1, c1)):
                        nc.tensor.matmul(
                            ps,
                            lhsT=wkvc[hc],
                            rhs=xt_t[hc][:, j * 512:(j + 1) * 512],
                            start=st, stop=sp, skip_group_check=True)
                proj_drain_qt(qt_sb, a0, 0, 0)
                proj_drain_qt(qt_sb, a1, 0, 1)
                proj_drain_qt(qt_sb, a2, 1, 0)
                proj_drain_kv(kvt_sb, c0, 0, 0)
                proj_drain_kv(kvt_sb, c1, 0, 1)
                a3 = psum.tile([P, 1024], F32, tag="A", bufs=2)
                c2 = psum.tile([P, 512], F32, tag="C", bufs=2)
                c3 = psum.tile([P, 512], F32, tag="C", bufs=2)
                for hc in range(HC):
                    st, sp = hc == 0, hc == HC - 1
                    for j in range(2):
                        nc.tensor.matmul(
                            a3[:, j * 512:(j + 1) * 512],
                            lhsT=wqc[hc][:, P:2 * P],
                            rhs=xt_t[hc][:, 1024 + j * 512:
                                         1024 + (j + 1) * 512],
                            start=st, stop=sp, skip_group_check=True)
                    for j, ps in ((0, c2), (1, c3)):
                        nc.tensor.matmul(
                            ps,
                            lhsT=wkvc[hc],
                            rhs=xt_t[hc][:, 1024 + j * 512:
                                         1024 + (j + 1) * 512],
                            start=st, stop=sp, skip_group_check=True)
                proj_drain_qt(qt_sb, a3, 1, 1)
                proj_drain_kv(kvt_sb, c2, 1, 0)
                proj_drain_kv(kvt_sb, c3, 1, 1)
                nc.sync.dma_start(bo_sb, bo_d)

            def make_proj_thunks(b, xt_t, qt_sb, kvt_sb, group):
                """Deferred projections for batch b, emitted as fillers.
                group: "kv" | "qc0" | "qc1" — kv first so K/V finish early."""
                if group == "kv":
                    chains = [("kv", None, sh, j) for sh in range(2)
                              for j in range(2)]
                else:
                    qc = int(group[2])
                    chains = [("qt", qc, sh, j) for sh in range(2)
                              for j in range(2)]

                def chain_thunks(kind, qc, sh, j):
                    ps_box = {}

                    def start_thunk():
                        ps_box["ps"] = psum.tile([P, 512], F32, tag="C", bufs=2,
                                                 name="projps")
                        return 0

                    def mk_mm(t):
                        def mm_thunk():
                            ps = ps_box["ps"]
                            for hc in (2 * t, 2 * t + 1):
                                st, sp = hc == 0, hc == HC - 1
                                if kind == "qt":
                                    nc.tensor.matmul(
                                        ps,
                                        lhsT=wqc[hc][:, qc * P:(qc + 1) * P],
                                        rhs=xt_t[hc][:, sh * 1024 + j * 512:
                                                     sh * 1024 + (j + 1) * 512],
                                        start=st, stop=sp,
                                        skip_group_check=True)
                                else:
                                    nc.tensor.matmul(
                                        ps,
                                        lhsT=wkvc[hc],
                                        rhs=xt_t[hc][:, sh * 1024 + j * 512:
                                                     sh * 1024 + (j + 1) * 512],
                                        start=st, stop=sp,
                                        skip_group_check=True)
                            return 2
                        return mm_thunk

                    def drain_thunk():
                        ps = ps_box["ps"]
                        if kind == "qt":
                            nc.vector.tensor_tensor(
                                out=qt_sb[:, qc, sh * 1024 + j * 512:
                                          sh * 1024 + (j + 1) * 512],
                                in0=ps,
                                in1=bq_sb[:, qc:qc + 1].to_broadcast((P, 512)),
                                op=mybir.AluOpType.add)
                        else:
                            proj_drain_kv(kvt_sb, ps, sh, j)
                        return 0

                    yield start_thunk
                    for t in range(HC // 2):
                        yield mk_mm(t)
                    yield drain_thunk

                for ch in chains:
                    for th in chain_thunks(*ch):
                        work.append(th)

            def finish_kv(b, kvt_sb):
                """K replica at partitions 64-127 + V transpose via XBAR."""
                kt2_sb = kvt_pool.tile([P, S], BF16, tag="kt2")
                nc.sync.dma_start(kt2_sb[64:128, :], kvt_sb[0:64, :])
                # XBAR transpose needs a contiguous destination; land V there
                # and splice the softmax-denominator ones column on the DVE.
                vpt = vp_pool.tile([P, SC, D], BF16, tag="vpt")
                nc.sync.dma_start_transpose(vpt, kvt_sb[64:128, :])
                vp_sb = vp_pool.tile([P, SC, 65], BF16, tag="vp")
                nc.vector.memset(vp_sb[:, :, 64], 1.0)
                nc.vector.tensor_copy(out=vp_sb[:, :, 0:64], in_=vpt)
                return kt2_sb, vp_sb

            # ---------------- out-projection chains ----------------
            # wo column-slices are loaded exactly once: jq0/jq1 into the wos
            # pool, jq2/jq3 into the xt chunk buffers (dead after batch-1
            # projections). wo_rhs resolves (jq, sq) -> AP at emission time.
            wo_parts = {}

            def wo_rhs(jq, sq):
                if jq < 2:
                    return wo_parts[jq][:, sq, :]
                t = wo_parts[(jq, sq // 8)]
                return t[:, (sq % 8) // 4,
                         (sq % 4) * 512:(sq % 4 + 1) * 512]

            def outproj_load(jq):
                def load_thunk(jq=jq):
                    if jq < 2:
                        w = wos_pool.tile([P, SC, 512], BF16, tag="wo",
                                          name="wosl")
                        nc.sync.dma_start(w, wo_d[:, jq])
                        wo_parts[jq] = w
                    else:
                        for q in range(2):
                            t = xt_pool.tile(
                                [P, 2, S], BF16,
                                tag=f"xt{(jq - 2) * 2 + q + 4}",
                                name="woxt")
                            nc.sync.dma_start(
                                t, wo_d[:, jq, 8 * q:8 * q + 8, :]
                                .rearrange("p (c x) j -> p c (x j)", c=2))
                            wo_parts[(jq, q)] = t
                    return 0
                work.append(load_thunk)

            def outproj_chain(b, half, rc, jq):
                """Thunks: one [128,512] row-chunk x col-slice chain."""
                box = {}

                def start_thunk():
                    box["ps"] = psum.tile([P, 512], F32, tag="C",
                                          bufs=2, name="outps")
                    return 0
                work.append(start_thunk)

                def mk_mm(t):
                    def mm_thunk():
                        for sq in (2 * t, 2 * t + 1):
                            nc.tensor.matmul(
                                box["ps"],
                                lhsT=half[:, sq, :],
                                rhs=wo_rhs(jq, sq),
                                start=sq == 0, stop=sq == SC - 1,
                                skip_group_check=True)
                        return 2
                    return mm_thunk
                for t in range(SC // 2):
                    work.append(mk_mm(t))

                def drain_thunk():
                    o = out_pool.tile([P, 512], F32, tag="out")
                    nc.vector.tensor_tensor(
                        out=o, in0=box["ps"],
                        in1=bo_sb[:, jq * 512:(jq + 1) * 512],
                        op=mybir.AluOpType.add)
                    nc.sync.dma_start(
                        out_d[b, rc * P:(rc + 1) * P,
                              jq * 512:(jq + 1) * 512], o)
                    return 0
                work.append(drain_thunk)

            # ---------------- attention ----------------
            def attn_phase(b, qt_sb, kvt_sb, kt2_sb, vp_sb, attn_halves,
                           gate_hook=None):
                pair_state = {}
                for h in range(4):
                    pbase = (h % 2) * 64
                    qt_h = qt_sb[pbase:pbase + 64, h // 2, :]
                    kt_h = (kvt_sb if h % 2 == 0 else kt2_sb)[
                        pbase:pbase + 64, :]
                    attnT_sb = attnT_pool.tile([65, S], BF16, tag="attnT")
                    araw = araw_pool.tile([P, SC, D], BF16,
                                          tag=f"araw{h % 2}")
                    if h % 2 == 0:
                        pair_state.clear()
                        den = dxt_pool.tile([16, S], BF16, tag="den")
                        nc.vector.memset(den, 1.0)
                        rcp0 = rcp_pool.tile([P, SC], F32, tag="rcp")
                        rcp1 = rcp_pool.tile([P, SC], F32, tag="rcp")
                        pair_state.update(den=den, araw0=araw,
                                          rcp0=rcp0, rcp1=rcp1)
                    else:
                        den = pair_state["den"]
                        pair_state["araw1"] = araw
                    for qtp in range(2):
                        av = psum.tile([P, 1024], F32, tag="av", bufs=1)
                        prev = None
                        for kc in range(SC):
                            sc_ps = psum.tile([P, 1024], F32, tag="A", bufs=2)
                            for j in range(2):
                                nc.tensor.matmul(
                                    sc_ps[:, j * 512:(j + 1) * 512],
                                    lhsT=kt_h[:, kc * P:(kc + 1) * P],
                                    rhs=qt_h[:, qtp * 1024 + j * 512:
                                             qtp * 1024 + (j + 1) * 512],
                                    start=True, stop=True)
                            if prev is not None:
                                ppt, pkc = prev
                                for j in range(2):
                                    nc.tensor.matmul(
                                        av[0:65, j * 512:(j + 1) * 512],
                                        lhsT=vp_sb[:, pkc, :],
                                        rhs=ppt[:, j * 512:(j + 1) * 512],
                                        start=(pkc == 0), stop=(pkc == SC - 1),
                                        skip_group_check=True)
                            if kc % 2 == 1 and not (b == 0 and h == 0
                                                    and qtp == 0):
                                pump(1 if h < 2 else 2)
                            pt = pt_pool.tile([P, 1024], BF16, tag="pt")
                            nc.scalar.activation(
                                pt, sc_ps, mybir.ActivationFunctionType.Exp)
                            prev = (pt, kc)
                        ppt, pkc = prev
                        for j in range(2):
                            nc.tensor.matmul(
                                av[0:65, j * 512:(j + 1) * 512],
                                lhsT=vp_sb[:, pkc, :],
                                rhs=ppt[:, j * 512:(j + 1) * 512],
                                start=(pkc == 0), stop=(pkc == SC - 1),
                                skip_group_check=True)
                        nc.vector.tensor_copy(
                            out=attnT_sb[:, qtp * 1024:(qtp + 1) * 1024],
                            in_=av[0:65, :])
                        # incremental transpose + normalize per q-half, all
                        # off the PE: attn rows via XBAR; the softmax
                        # denominator row (64) via a 16-partition-aligned
                        # collect tile (XBAR offset rule), one per head-pair.
                        hw = slice(qtp * 1024, (qtp + 1) * 1024)
                        cw = slice(qtp * 8, (qtp + 1) * 8)
                        nc.sync.dma_start_transpose(
                            araw[:, cw, :], attnT_sb[0:64, hw])
                        nc.sync.dma_start(den[h % 2:h % 2 + 1, hw],
                                          attnT_sb[64:65, hw])
                        if h % 2 == 1:
                            dxt = dxt_pool.tile([P, 8, 16], BF16, tag="dxt")
                            nc.sync.dma_start_transpose(dxt, den[:, hw])
                            half = attn_halves[h // 2]
                            for hh in range(2):
                                rcp = pair_state[f"rcp{hh}"]
                                nc.vector.reciprocal(rcp[:, cw],
                                                     dxt[:, :, hh])
                                ar = pair_state[f"araw{hh}"]
                                eng = nc.vector if hh == 0 else nc.gpsimd
                                for tb in range(qtp * 8, qtp * 8 + 8):
                                    eng.tensor_tensor(
                                        out=half[:, tb,
                                                 hh * D:(hh + 1) * D],
                                        in0=ar[:, tb, :],
                                        in1=rcp[:, tb:tb + 1]
                                        .to_broadcast((P, D)),
                                        op=mybir.AluOpType.mult)
                        pump(3)
                    if gate_hook is not None:
                        gate_hook(h)

            # ================= schedule =================
            # batch 0 projections, paced against the xt chunk DMAs
            xt0 = load_xt(0)
            qt0 = qt_pool.tile([P, 2, S], BF16, tag="qt")
            kvt0 = kvt_pool.tile([P, S], BF16, tag="kvt")
            proj_phase_b0(xt0, qt0, kvt0)
            kt20, vp0 = finish_kv(0, kvt0)

            # batch 1 xt load starts as soon as batch 0 mms release chunks
            xt1 = load_xt(1)
            qt1 = qt_pool.tile([P, 2, S], BF16, tag="qt")
            kvt1 = kvt_pool.tile([P, S], BF16, tag="kvt")
            kv1_box = {}

            def finish_kv1_thunk():
                kv1_box["r"] = finish_kv(1, kvt1)
                return 0
            make_proj_thunks(1, xt1, qt1, kvt1, "kv")
            work.append(finish_kv1_thunk)
            make_proj_thunks(1, xt1, qt1, kvt1, "qc0")
            make_proj_thunks(1, xt1, qt1, kvt1, "qc1")
            # wo jq2/jq3 slices into the freed xt chunk buffers
            outproj_load(2)
            outproj_load(3)

            attn0 = [attn_pool.tile([P, SC, P], BF16, tag=f"attn{i}",
                                     name=f"attn0_{i}") for i in range(2)]
            attn1 = [attn_pool.tile([P, SC, P], BF16, tag=f"attn{i}",
                                     name=f"attn1_{i}") for i in range(2)]

            # batch 0 attention, consuming batch 1 projection fillers
            attn_phase(0, qt0, kvt0, kt20, vp0, attn0)
            flush()
            kt21, vp1 = kv1_box["r"]

            # batch 1 attention, consuming out-projection fillers:
            # batch 0 rows first; batch-1 heads 0/1 rows as soon as ready
            outproj_load(0)
            outproj_load(1)
            for jq in range(4):
                outproj_chain(0, attn0[0], 0, jq)
                outproj_chain(0, attn0[1], 1, jq)

            def gate(h):
                if h == 1:
                    for jq in range(4):
                        outproj_chain(1, attn1[0], 0, jq)

            attn_phase(1, qt1, kvt1, kt21, vp1, attn1, gate_hook=gate)
            for jq in range(4):
                outproj_chain(1, attn1[1], 1, jq)
            flush()

    nc.compile()
    return nc


def _get_nc():
    if "nc" not in _CACHE:
        _CACHE["nc"] = _build()
    return _CACHE["nc"]


def _prep_inputs(hidden_state, w_q, b_q, w_k, b_k, w_v, b_v, w_o, b_o):
    """Host-side sharding/layout prep. Only layout/dtype transforms."""
    xt = np.ascontiguousarray(hidden_state.transpose(0, 2, 1)).astype(NP_BF16)
    # wo pre-arranged to [P, 4, SC, 512] so each column-slice DMA line is
    # one contiguous 16KB run per partition
    wo = np.ascontiguousarray(
        w_o.reshape(SC, P, 4, 512).transpose(1, 2, 0, 3)).astype(NP_BF16)
    bo = np.broadcast_to(b_o.astype(np.float32), (P, HID)).copy()
    in_maps = []
    for g in range(NCORES):
        wq_g = np.ascontiguousarray(
            (w_q[:, g * QF:(g + 1) * QF] * 0.125)
            .reshape(HC, P, QF).transpose(1, 0, 2)).astype(NP_BF16)
        bq_g = np.ascontiguousarray(
            (b_q[g * QF:(g + 1) * QF] * 0.125).reshape(2, P)).astype(np.float32)
        wkv_g = np.ascontiguousarray(np.concatenate(
            [w_k[:, g * D:(g + 1) * D], w_v[:, g * D:(g + 1) * D]],
            axis=1).reshape(HC, P, P).transpose(1, 0, 2)).astype(NP_BF16)
        bkv_g = np.ascontiguousarray(np.concatenate(
            [b_k[g * D:(g + 1) * D], b_v[g * D:(g + 1) * D]])
            .reshape(P, 1)).astype(np.float32)
        in_maps.append({
            "xt": xt, "wq": wq_g, "bq": bq_g, "wkv": wkv_g, "bkv": bkv_g,
            "wo": wo, "bo": bo,
        })
    return in_maps


def kernel(hidden_state, w_q, b_q, w_k, b_k, w_v, b_v, w_o, b_o,
           _trace=False):
    hidden_state = np.asarray(hidden_state, np.float32)
    args = [np.asarray(a, np.float32) for a in
            (w_q, b_q, w_k, b_k, w_v, b_v, w_o, b_o)]
    nc = _get_nc()
    in_maps = _prep_inputs(hidden_state, *args)
    res = bass_utils.run_bass_kernel_spmd(
        nc, in_maps, core_ids=list(range(NCORES)), trace=_trace)
    out = np.concatenate([res.results[g]["out"] for g in range(NCORES)],
                         axis=1).astype(np.float32)
    if _trace:
        _CACHE["last_results"] = res
    return out


# revision 27
# speedup vs baseline: 1.0001x; 1.0001x over previous
"""GQA kernel for Trainium2, sharded over 8 NeuronCores.

Sharding: tensor-parallel over heads. Core g owns Q heads 4g..4g+3 and KV
group g (GQA rep=4, so all 4 local heads share one K/V). The reference's
final projection contracts over the *sequence* axis (faithful swapaxes
quirk), so output rows partition cleanly by head: core g produces rows
g*256..(g+1)*256 of the [2, 2048, 2048] output. No collectives.

v2 schedule (software-pipelined, trace-driven):
  - xt is loaded per-hidden-chunk; projection matmuls start as chunks land.
  - attention inner loop is ACT(exp)-bound; batch-1 projection and batch-0
    out-projection matmul chains are injected between attention matmuls as
    "filler" thunks so the PE never idles while ACT chews exp tiles.
  - all transposes (V, attn, softmax denominator) run on the DMA XBAR
    (dma_start_transpose), not the PE.
  - out-projection is split per head-pair so rows for heads 0/1 of batch 1
    are projected while heads 2/3 still run attention; only the last
    quarter of the out-projection remains after the attention pipeline.
"""
import numpy as np
import ml_dtypes
from collections import deque

import concourse.bass as bass
import concourse.bacc as bacc
import concourse.mybir as mybir
import concourse.tile as tile
from concourse import bass_utils

BF16 = mybir.dt.bfloat16
F32 = mybir.dt.float32
NP_BF16 = ml_dtypes.bfloat16

B, S, HID = 2, 2048, 2048
NCORES = 8
HEADS_PER_CORE = 4   # of 32
D = 64               # head dim
QF = HEADS_PER_CORE * D   # 256 q-features per core
P = 128
HC = HID // P        # 16 hidden chunks
SC = S // P          # 16 seq chunks

_CACHE = {}


def _build():
    nc = bacc.Bacc("TRN2", target_bir_lowering=False, debug=False,
                   num_devices=NCORES)
    # ---- DRAM I/O ----
    xt_d = nc.dram_tensor("xt", [B, HID, S], BF16, kind="ExternalInput").ap()
    # weights arrive pre-arranged on host so every DMA line is contiguous
    wq_d = nc.dram_tensor("wq", [P, HC, QF], BF16, kind="ExternalInput").ap()
    bq_d = nc.dram_tensor("bq", [2, P], F32, kind="ExternalInput").ap()
    wkv_d = nc.dram_tensor("wkv", [P, HC, P], BF16, kind="ExternalInput").ap()
    bkv_d = nc.dram_tensor("bkv", [P, 1], F32, kind="ExternalInput").ap()
    wo_d = nc.dram_tensor("wo", [P, 4, SC, 512], BF16,
                          kind="ExternalInput").ap()
    bo_d = nc.dram_tensor("bo", [P, HID], F32, kind="ExternalInput").ap()
    out_d = nc.dram_tensor("out", [B, QF, HID], F32, kind="ExternalOutput").ap()

    with tile.TileContext(nc) as tc:
        with (
            tc.tile_pool(name="consts", bufs=1) as consts,
            tc.tile_pool(name="xt", bufs=1) as xt_pool,
            tc.tile_pool(name="qt", bufs=2) as qt_pool,
            tc.tile_pool(name="kvt", bufs=2) as kvt_pool,
            tc.tile_pool(name="vp", bufs=2) as vp_pool,
            tc.tile_pool(name="pt", bufs=2) as pt_pool,
            tc.tile_pool(name="attnT", bufs=2) as attnT_pool,
            tc.tile_pool(name="araw", bufs=2) as araw_pool,
            tc.tile_pool(name="dxt", bufs=2) as dxt_pool,
            tc.tile_pool(name="rcp", bufs=2) as rcp_pool,
            tc.tile_pool(name="attn", bufs=2) as attn_pool,
            tc.tile_pool(name="wos", bufs=2) as wos_pool,
            tc.tile_pool(name="outp", bufs=2) as out_pool,
            tc.tile_pool(name="psum", bufs=1, space="PSUM") as psum,
        ):
            # ---- constants (bo deferred: not needed until out-proj) ----
            bq_sb = consts.tile([P, 2], F32)
            nc.sync.dma_start(bq_sb, bq_d.rearrange("c p -> p c"))
            bkv_sb = consts.tile([P, 1], F32)
            nc.sync.dma_start(bkv_sb, bkv_d)
            wq_sb = consts.tile([P, HC, QF], BF16)
            nc.sync.dma_start(wq_sb, wq_d)
            wkv_sb = consts.tile([P, HC, P], BF16)
            nc.sync.dma_start(wkv_sb, wkv_d)
            wqc = [wq_sb[:, hc] for hc in range(HC)]
            wkvc = [wkv_sb[:, hc] for hc in range(HC)]
            bo_sb = consts.tile([P, HID], F32)

            # filler thunk queue: each thunk emits a small chunk of deferred
            # PE work (or a DMA, cost 0) and returns its PE cost in matmuls.
            work = deque()

            def pump(budget=1):
                spent = 0
                while work and spent < budget:
                    spent += work.popleft()()

            def flush():
                while work:
                    work.popleft()()

            # ---------------- projections ----------------
            def load_xt(b):
                """xt in chunk-pairs: fewer triggers, 8KB DMA lines."""
                pairs = []
                for hp in range(HC // 2):
                    t = xt_pool.tile([P, 2, S], BF16, tag=f"xt{hp}",
                                     name=f"xtc{hp}")
                    nc.sync.dma_start(
                        t, xt_d[b, hp * 2 * P:(hp + 1) * 2 * P, :]
                        .rearrange("(c p) s -> p c s", p=P))
                    pairs.append(t)
                return [pairs[hc // 2][:, hc % 2, :] for hc in range(HC)]

            def proj_drain_qt(qt_sb, ps, qc, sh):
                nc.vector.tensor_tensor(
                    out=qt_sb[:, qc, sh * 1024:(sh + 1) * 1024],
                    in0=ps, in1=bq_sb[:, qc:qc + 1].to_broadcast((P, 1024)),
                    op=mybir.AluOpType.add)

            def proj_drain_kv(kvt_sb, ps, sh, j):
                nc.vector.tensor_tensor(
                    out=kvt_sb[:, sh * 1024 + j * 512:sh * 1024 + (j + 1) * 512],
                    in0=ps, in1=bkv_sb[:, 0:1].to_broadcast((P, 512)),
                    op=mybir.AluOpType.add)

            def proj_phase_b0(xt_t, qt_sb, kvt_sb):
                """Paced projection for batch 0: matmuls chase the xt DMAs.
                Round 1 (DMA-paced): qc0 both halves, qc1 sh0, kv sh0.
                Round 2 (chunks resident): qc1 sh1, kv sh1."""
                a0 = psum.tile([P, 1024], F32, tag="A", bufs=2)
                a1 = psum.tile([P, 1024], F32, tag="A", bufs=2)
                a2 = psum.tile([P, 1024], F32, tag="av", bufs=1)
                c0 = psum.tile([P, 512], F32, tag="C", bufs=2)
                c1 = psum.tile([P, 512], F32, tag="C", bufs=2)
                for hc in range(HC):
                    st, sp = hc == 0, hc == HC - 1
                    for qc, sh, ps in ((0, 0, a0), (0, 1, a1), (1, 0, a2)):
                        for j in range(2):
                            nc.tensor.matmul(
                                ps[:, j * 512:(j + 1) * 512],
                                lhsT=wqc[hc][:, qc * P:(qc + 1) * P],
                                rhs=xt_t[hc][:, sh * 1024 + j * 512:
                                             sh * 1024 + (j + 1) * 512],
                                start=st, stop=sp, skip_group_check=True)
                    for j, ps in ((0, c0), (1, c1)):
                        nc.tensor.matmul(
                            ps,
                            lhsT=wkvc[hc],
                            rhs=xt_t[hc][:, j * 512:(j + 1) * 512],
                            start=st, stop=sp, skip_group_check=True)
                proj_drain_qt(qt_sb, a0, 0, 0)
                proj_drain_qt(qt_sb, a1, 0, 1)
                proj_drain_qt(qt_sb, a2, 1, 0)
                proj_drain_kv(kvt_sb, c0, 0, 0)
                proj_drain_kv(kvt_sb, c1, 0, 1)
                a3 = psum.tile([P, 1024], F32, tag="A", bufs=2)
                c2 = psum.tile([P, 512], F32, tag="C", bufs=2)
                c3 = psum.tile([P, 512], F32, tag="C", bufs=2)
                for hc in range(HC):
                    st, sp = hc == 0, hc == HC - 1
                    for j in range(2):
                        nc.tensor.matmul(
                            a3[:, j * 512:(j + 1) * 512],
                            lhsT=wqc[hc][:, P:2 * P],
                            rhs=xt_t[hc][:, 1024 + j * 512:
                                         1024 + (j + 1) * 512],
                            start=st, stop=sp, skip_group_check=True)
                    for j, ps in ((0, c2), (1, c3)):
                        nc.tensor.matmul(
                            ps,
                            lhsT=wkvc[hc],
                            rhs=xt_t[hc][:, 1024 + j * 512:
                                         1024 + (j + 1) * 512],
                            start=st, stop=sp, skip_group_check=True)
                proj_drain_qt(qt_sb, a3, 1, 1)
                proj_drain_kv(kvt_sb, c2, 1, 0)
                proj_drain_kv(kvt_sb, c3, 1, 1)
                nc.sync.dma_start(bo_sb, bo_d)

            def make_proj_thunks(b, xt_t, qt_sb, kvt_sb, group):
                """Deferred projections for batch b, emitted as fillers.
                group: "kv" | "qc0" | "qc1" — kv first so K/V finish early."""
                if group == "kv":
                    chains = [("kv", None, sh, j) for sh in range(2)
                              for j in range(2)]
                else:
                    qc = int(group[2])
                    chains = [("qt", qc, sh, j) for sh in range(2)
                              for j in range(2)]

                def chain_thunks(kind, qc, sh, j):
                    ps_box = {}

                    def start_thunk():
                        ps_box["ps"] = psum.tile([P, 512], F32, tag="C", bufs=2,
                                                 name="projps")
                        return 0

                    def mk_mm(t):
                        def mm_thunk():
                            ps = ps_box["ps"]
                            for hc in (2 * t, 2 * t + 1):
                                st, sp = hc == 0, hc == HC - 1
                                if kind == "qt":
                                    nc.tensor.matmul(
                                        ps,
                                        lhsT=wqc[hc][:, qc * P:(qc + 1) * P],
                                        rhs=xt_t[hc][:, sh * 1024 + j * 512:
                                                     sh * 1024 + (j + 1) * 512],
                                        start=st, stop=sp,
                                        skip_group_check=True)
                                else:
                                    nc.tensor.matmul(
                                        ps,
                                        lhsT=wkvc[hc],
                                        rhs=xt_t[hc][:, sh * 1024 + j * 512:
                                                     sh * 1024 + (j + 1) * 512],
                                        start=st, stop=sp,
                                        skip_group_check=True)
                            return 2
                        return mm_thunk

                    def drain_thunk():
                        ps = ps_box["ps"]
                        if kind == "qt":
                            nc.vector.tensor_tensor(
                                out=qt_sb[:, qc, sh * 1024 + j * 512:
                                          sh * 1024 + (j + 1) * 512],
                                in0=ps,
                                in1=bq_sb[:, qc:qc + 1].to_broadcast((P, 512)),
                                op=mybir.AluOpType.add)
                        else:
                            proj_drain_kv(kvt_sb, ps, sh, j)
                        return 0

                    yield start_thunk
                    for t in range(HC // 2):
                        yield mk_mm(t)
                    yield drain_thunk

                for ch in chains:
                    for th in chain_thunks(*ch):
                        work.append(th)

            def finish_kv(b, kvt_sb):
                """K replica at partitions 64-127 + V transpose via XBAR."""
                kt2_sb = kvt_pool.tile([P, S], BF16, tag="kt2")
                nc.sync.dma_start(kt2_sb[64:128, :], kvt_sb[0:64, :])
                # XBAR transpose needs a contiguous destination; land V there
                # and splice the softmax-denominator ones column on the DVE.
                vpt = vp_pool.tile([P, SC, D], BF16, tag="vpt")
                nc.sync.dma_start_transpose(vpt, kvt_sb[64:128, :])
                vp_sb = vp_pool.tile([P, SC, 65], BF16, tag="vp")
                nc.vector.memset(vp_sb[:, :, 64], 1.0)
                nc.vector.tensor_copy(out=vp_sb[:, :, 0:64], in_=vpt)
                return kt2_sb, vp_sb

            # ---------------- out-projection chains ----------------
            # wo column-slices are loaded exactly once: jq0/jq1 into the wos
            # pool, jq2/jq3 into the xt chunk buffers (dead after batch-1
            # projections). wo_rhs resolves (jq, sq) -> AP at emission time.
            wo_parts = {}

            def wo_rhs(jq, sq):
                if jq < 2:
                    return wo_parts[jq][:, sq, :]
                t = wo_parts[(jq, sq // 8)]
                return t[:, (sq % 8) // 4,
                         (sq % 4) * 512:(sq % 4 + 1) * 512]

            def outproj_load(jq):
                def load_thunk(jq=jq):
                    if jq < 2:
                        w = wos_pool.tile([P, SC, 512], BF16, tag="wo",
                                          name="wosl")
                        nc.sync.dma_start(w, wo_d[:, jq])
                        wo_parts[jq] = w
                    else:
                        for q in range(2):
                            t = xt_pool.tile(
                                [P, 2, S], BF16,
                                tag=f"xt{(jq - 2) * 2 + q + 4}",
                                name="woxt")
                            nc.sync.dma_start(
                                t, wo_d[:, jq, 8 * q:8 * q + 8, :]
                                .rearrange("p (c x) j -> p c (x j)", c=2))
                            wo_parts[(jq, q)] = t
                    return 0
                work.append(load_thunk)

            def outproj_chain(b, half, rc, jq):
                """Thunks: one [128,512] row-chunk x col-slice chain."""
                box = {}

                def start_thunk():
                    box["ps"] = psum.tile([P, 512], F32, tag="C",
                                          bufs=2, name="outps")
                    return 0
                work.append(start_thunk)

                def mk_mm(t):
                    def mm_thunk():
                        for sq in (2 * t, 2 * t + 1):
                            nc.tensor.matmul(
                                box["ps"],
                                lhsT=half[:, sq, :],
                                rhs=wo_rhs(jq, sq),
                                start=sq == 0, stop=sq == SC - 1,
                                skip_group_check=True)
                        return 2
                    return mm_thunk
                for t in range(SC // 2):
                    work.append(mk_mm(t))

                def drain_thunk():
                    o = out_pool.tile([P, 512], F32, tag="out")
                    nc.vector.tensor_tensor(
                        out=o, in0=box["ps"],
                        in1=bo_sb[:, jq * 512:(jq + 1) * 512],
                        op=mybir.AluOpType.add)
                    nc.sync.dma_start(
                        out_d[b, rc * P:(rc + 1) * P,
                              jq * 512:(jq + 1) * 512], o)
                    return 0
                work.append(drain_thunk)

            # ---------------- attention ----------------
            def attn_phase(b, qt_sb, kvt_sb, kt2_sb, vp_sb, attn_halves,
                           gate_hook=None):
                pair_state = {}
                for h in range(4):
                    pbase = (h % 2) * 64
                    qt_h = qt_sb[pbase:pbase + 64, h // 2, :]
                    kt_h = (kvt_sb if h % 2 == 0 else kt2_sb)[
                        pbase:pbase + 64, :]
                    attnT_sb = attnT_pool.tile([65, S], BF16, tag="attnT")
                    araw = araw_pool.tile([P, SC, D], BF16,
                                          tag=f"araw{h % 2}")
                    if h % 2 == 0:
                        pair_state.clear()
                        den = dxt_pool.tile([16, S], BF16, tag="den")
                        nc.vector.memset(den, 1.0)
                        rcp0 = rcp_pool.tile([P, SC], F32, tag="rcp")
                        rcp1 = rcp_pool.tile([P, SC], F32, tag="rcp")
                        pair_state.update(den=den, araw0=araw,
                                          rcp0=rcp0, rcp1=rcp1)
                    else:
                        den = pair_state["den"]
                        pair_state["araw1"] = araw
                    for qtp in range(2):
                        av = psum.tile([P, 1024], F32, tag="av", bufs=1)
                        prev = None
                        for kc in range(SC):
                            sc_ps = psum.tile([P, 1024], F32, tag="A", bufs=2)
                            for j in range(2):
                                nc.tensor.matmul(
                                    sc_ps[:, j * 512:(j + 1) * 512],
                                    lhsT=kt_h[:, kc * P:(kc + 1) * P],
                                    rhs=qt_h[:, qtp * 1024 + j * 512:
                                             qtp * 1024 + (j + 1) * 512],
                                    start=True, stop=True)
                            if prev is not None:
                                ppt, pkc = prev
                                for j in range(2):
                                    nc.tensor.matmul(
                                        av[0:65, j * 512:(j + 1) * 512],
                                        lhsT=vp_sb[:, pkc, :],
                                        rhs=ppt[:, j * 512:(j + 1) * 512],
                                        start=(pkc == 0), stop=(pkc == SC - 1),
                                        skip_group_check=True)
                            if kc % 2 == 1 and not (b == 0 and h == 0
                                                    and qtp == 0):
                                pump(1 if h < 2 else 2)
                            pt = pt_pool.tile([P, 1024], BF16, tag="pt")
                            nc.scalar.activation(
                                pt, sc_ps, mybir.ActivationFunctionType.Exp)
                            prev = (pt, kc)
                        ppt, pkc = prev
                        for j in range(2):
                            nc.tensor.matmul(
                                av[0:65, j * 512:(j + 1) * 512],
                                lhsT=vp_sb[:, pkc, :],
                                rhs=ppt[:, j * 512:(j + 1) * 512],
                                start=(pkc == 0), stop=(pkc == SC - 1),
                                skip_group_check=True)
                        nc.scalar.copy(
                            attnT_sb[:, qtp * 1024:(qtp + 1) * 1024],
                            av[0:65, :])
                        # incremental transpose + normalize per q-half, all
                        # off the PE: attn rows via XBAR; the softmax
                        # denominator row (64) via a 16-partition-aligned
                        # collect tile (XBAR offset rule), one per head-pair.
                        hw = slice(qtp * 1024, (qtp + 1) * 1024)
                        cw = slice(qtp * 8, (qtp + 1) * 8)
                        nc.sync.dma_start_transpose(
                            araw[:, cw, :], attnT_sb[0:64, hw])
                        nc.sync.dma_start(den[h % 2:h % 2 + 1, hw],
                                          attnT_sb[64:65, hw])
                        if h % 2 == 1:
                            dxt = dxt_pool.tile([P, 8, 16], BF16, tag="dxt")
                            nc.sync.dma_start_transpose(dxt, den[:, hw])
                            half = attn_halves[h // 2]
                            for hh in range(2):
                                rcp = pair_state[f"rcp{hh}"]
                                nc.vector.reciprocal(rcp[:, cw],
                                                     dxt[:, :, hh])
                                ar = pair_state[f"araw{hh}"]
                                eng = nc.vector if hh == 0 else nc.gpsimd
                                for tb in range(qtp * 8, qtp * 8 + 8):
                                    eng.tensor_tensor(
                                        out=half[:, tb,
                                                 hh * D:(hh + 1) * D],
                                        in0=ar[:, tb, :],
                                        in1=rcp[:, tb:tb + 1]
                                        .to_broadcast((P, D)),
                                        op=mybir.AluOpType.mult)
                        pump(3)
                    if gate_hook is not None:
                        gate_hook(h)

            # ================= schedule =================
            # batch 0 projections, paced against the xt chunk DMAs
            xt0 = load_xt(0)
            qt0 = qt_pool.tile([P, 2, S], BF16, tag="qt")
            kvt0 = kvt_pool.tile([P, S], BF16, tag="kvt")
            proj_phase_b0(xt0, qt0, kvt0)
            kt20, vp0 = finish_kv(0, kvt0)

            # batch 1 xt load starts as soon as batch 0 mms release chunks
            xt1 = load_xt(1)
            qt1 = qt_pool.tile([P, 2, S], BF16, tag="qt")
            kvt1 = kvt_pool.tile([P, S], BF16, tag="kvt")
            kv1_box = {}

            def finish_kv1_thunk():
                kv1_box["r"] = finish_kv(1, kvt1)
                return 0
            make_proj_thunks(1, xt1, qt1, kvt1, "kv")
            work.append(finish_kv1_thunk)
            make_proj_thunks(1, xt1, qt1, kvt1, "qc0")
            make_proj_thunks(1, xt1, qt1, kvt1, "qc1")
            # wo jq2/jq3 slices into the freed xt chunk buffers
            outproj_load(2)
            outproj_load(3)

            attn0 = [attn_pool.tile([P, SC, P], BF16, tag=f"attn{i}",
                                     name=f"attn0_{i}") for i in range(2)]
            attn1 = [attn_pool.tile([P, SC, P], BF16, tag=f"attn{i}",
                                     name=f"attn1_{i}") for i in range(2)]

            # batch 0 attention, consuming batch 1 projection fillers
            attn_phase(0, qt0, kvt0, kt20, vp0, attn0)
            flush()
            kt21, vp1 = kv1_box["r"]

            # batch 1 attention, consuming out-projection fillers:
            # batch 0 rows first; batch-1 heads 0/1 rows as soon as ready
            outproj_load(0)
            outproj_load(1)
            for jq in range(4):
                outproj_chain(0, attn0[0], 0, jq)
                outproj_chain(0, attn0[1], 1, jq)

            def gate(h):
                if h == 1:
                    for jq in range(4):
                        outproj_chain(1, attn1[0], 0, jq)

            attn_phase(1, qt1, kvt1, kt21, vp1, attn1, gate_hook=gate)
            for jq in range(4):
                outproj_chain(1, attn1[1], 1, jq)
            flush()

    nc.compile()
    return nc


def _get_nc():
    if "nc" not in _CACHE:
        _CACHE["nc"] = _build()
    return _CACHE["nc"]


def _prep_inputs(hidden_state, w_q, b_q, w_k, b_k, w_v, b_v, w_o, b_o):
    """Host-side sharding/layout prep. Only layout/dtype transforms."""
    xt = np.ascontiguousarray(hidden_state.transpose(0, 2, 1)).astype(NP_BF16)
    # wo pre-arranged to [P, 4, SC, 512] so each column-slice DMA line is
    # one contiguous 16KB run per partition
    wo = np.ascontiguousarray(
        w_o.reshape(SC, P, 4, 512).transpose(1, 2, 0, 3)).astype(NP_BF16)
    bo = np.broadcast_to(b_o.astype(np.float32), (P, HID)).copy()
    in_maps = []
    for g in range(NCORES):
        wq_g = np.ascontiguousarray(
            (w_q[:, g * QF:(g + 1) * QF] * 0.125)
            .reshape(HC, P, QF).transpose(1, 0, 2)).astype(NP_BF16)
        bq_g = np.ascontiguousarray(
            (b_q[g * QF:(g + 1) * QF] * 0.125).reshape(2, P)).astype(np.float32)
        wkv_g = np.ascontiguousarray(np.concatenate(
            [w_k[:, g * D:(g + 1) * D], w_v[:, g * D:(g + 1) * D]],
            axis=1).reshape(HC, P, P).transpose(1, 0, 2)).astype(NP_BF16)
        bkv_g = np.ascontiguousarray(np.concatenate(
            [b_k[g * D:(g + 1) * D], b_v[g * D:(g + 1) * D]])
            .reshape(P, 1)).astype(np.float32)
        in_maps.append({
            "xt": xt, "wq": wq_g, "bq": bq_g, "wkv": wkv_g, "bkv": bkv_g,
            "wo": wo, "bo": bo,
        })
    return in_maps


def kernel(hidden_state, w_q, b_q, w_k, b_k, w_v, b_v, w_o, b_o,
           _trace=False):
    hidden_state = np.asarray(hidden_state, np.float32)
    args = [np.asarray(a, np.float32) for a in
            (w_q, b_q, w_k, b_k, w_v, b_v, w_o, b_o)]
    nc = _get_nc()
    in_maps = _prep_inputs(hidden_state, *args)
    res = bass_utils.run_bass_kernel_spmd(
        nc, in_maps, core_ids=list(range(NCORES)), trace=_trace)
    out = np.concatenate([res.results[g]["out"] for g in range(NCORES)],
                         axis=1).astype(np.float32)
    if _trace:
        _CACHE["last_results"] = res
    return out


# revision 28
# speedup vs baseline: 1.0055x; 1.0055x over previous
"""GQA kernel for Trainium2, sharded over 8 NeuronCores.

Sharding: tensor-parallel over heads. Core g owns Q heads 4g..4g+3 and KV
group g (GQA rep=4, so all 4 local heads share one K/V). The reference's
final projection contracts over the *sequence* axis (faithful swapaxes
quirk), so output rows partition cleanly by head: core g produces rows
g*256..(g+1)*256 of the [2, 2048, 2048] output. No collectives.

v2 schedule (software-pipelined, trace-driven):
  - xt is loaded per-hidden-chunk; projection matmuls start as chunks land.
  - attention inner loop is ACT(exp)-bound; batch-1 projection and batch-0
    out-projection matmul chains are injected between attention matmuls as
    "filler" thunks so the PE never idles while ACT chews exp tiles.
  - all transposes (V, attn, softmax denominator) run on the DMA XBAR
    (dma_start_transpose), not the PE.
  - out-projection is split per head-pair so rows for heads 0/1 of batch 1
    are projected while heads 2/3 still run attention; only the last
    quarter of the out-projection remains after the attention pipeline.
"""
import numpy as np
import ml_dtypes
from collections import deque

import concourse.bass as bass
import concourse.bacc as bacc
import concourse.mybir as mybir
import concourse.tile as tile
from concourse import bass_utils

BF16 = mybir.dt.bfloat16
F32 = mybir.dt.float32
NP_BF16 = ml_dtypes.bfloat16

B, S, HID = 2, 2048, 2048
NCORES = 8
HEADS_PER_CORE = 4   # of 32
D = 64               # head dim
QF = HEADS_PER_CORE * D   # 256 q-features per core
P = 128
HC = HID // P        # 16 hidden chunks
SC = S // P          # 16 seq chunks

_CACHE = {}


def _build():
    nc = bacc.Bacc("TRN2", target_bir_lowering=False, debug=False,
                   num_devices=NCORES)
    # ---- DRAM I/O ----
    xt_d = nc.dram_tensor("xt", [B, HID, S], BF16, kind="ExternalInput").ap()
    # weights arrive pre-arranged on host so every DMA line is contiguous
    wq_d = nc.dram_tensor("wq", [P, HC, QF], BF16, kind="ExternalInput").ap()
    bq_d = nc.dram_tensor("bq", [2, P], F32, kind="ExternalInput").ap()
    wkv_d = nc.dram_tensor("wkv", [P, HC, P], BF16, kind="ExternalInput").ap()
    bkv_d = nc.dram_tensor("bkv", [P, 1], F32, kind="ExternalInput").ap()
    wo_d = nc.dram_tensor("wo", [P, 4, SC, 512], BF16,
                          kind="ExternalInput").ap()
    bo_d = nc.dram_tensor("bo", [P, HID], F32, kind="ExternalInput").ap()
    out_d = nc.dram_tensor("out", [B, QF, HID], F32, kind="ExternalOutput").ap()

    with tile.TileContext(nc) as tc:
        with (
            tc.tile_pool(name="consts", bufs=1) as consts,
            tc.tile_pool(name="xt", bufs=1) as xt_pool,
            tc.tile_pool(name="qt", bufs=2) as qt_pool,
            tc.tile_pool(name="kvt", bufs=2) as kvt_pool,
            tc.tile_pool(name="vp", bufs=2) as vp_pool,
            tc.tile_pool(name="pt", bufs=2) as pt_pool,
            tc.tile_pool(name="attnT", bufs=2) as attnT_pool,
            tc.tile_pool(name="araw", bufs=2) as araw_pool,
            tc.tile_pool(name="dxt", bufs=2) as dxt_pool,
            tc.tile_pool(name="rcp", bufs=2) as rcp_pool,
            tc.tile_pool(name="attn", bufs=2) as attn_pool,
            tc.tile_pool(name="wos", bufs=2) as wos_pool,
            tc.tile_pool(name="outp", bufs=2) as out_pool,
            tc.tile_pool(name="psum", bufs=1, space="PSUM") as psum,
        ):
            # ---- constants (bo deferred: not needed until out-proj) ----
            bq_sb = consts.tile([P, 2], F32)
            nc.sync.dma_start(bq_sb, bq_d.rearrange("c p -> p c"))
            bkv_sb = consts.tile([P, 1], F32)
            nc.sync.dma_start(bkv_sb, bkv_d)
            wq_sb = consts.tile([P, HC, QF], BF16)
            nc.sync.dma_start(wq_sb, wq_d)
            wkv_sb = consts.tile([P, HC, P], BF16)
            nc.sync.dma_start(wkv_sb, wkv_d)
            wqc = [wq_sb[:, hc] for hc in range(HC)]
            wkvc = [wkv_sb[:, hc] for hc in range(HC)]
            bo_sb = consts.tile([P, HID], F32)

            # filler thunk queue: each thunk emits a small chunk of deferred
            # PE work (or a DMA, cost 0) and returns its PE cost in matmuls.
            work = deque()

            def pump(budget=1):
                spent = 0
                while work and spent < budget:
                    spent += work.popleft()()

            def flush():
                while work:
                    work.popleft()()

            # ---------------- projections ----------------
            def load_xt(b):
                """xt in chunk-pairs: fewer triggers, 8KB DMA lines."""
                pairs = []
                for hp in range(HC // 2):
                    t = xt_pool.tile([P, 2, S], BF16, tag=f"xt{hp}",
                                     name=f"xtc{hp}")
                    nc.sync.dma_start(
                        t, xt_d[b, hp * 2 * P:(hp + 1) * 2 * P, :]
                        .rearrange("(c p) s -> p c s", p=P))
                    pairs.append(t)
                return [pairs[hc // 2][:, hc % 2, :] for hc in range(HC)]

            def proj_drain_qt(qt_sb, ps, qc, sh):
                nc.vector.tensor_tensor(
                    out=qt_sb[:, qc, sh * 1024:(sh + 1) * 1024],
                    in0=ps, in1=bq_sb[:, qc:qc + 1].to_broadcast((P, 1024)),
                    op=mybir.AluOpType.add)

            def proj_drain_kv(kvt_sb, ps, sh, j):
                nc.vector.tensor_tensor(
                    out=kvt_sb[:, sh * 1024 + j * 512:sh * 1024 + (j + 1) * 512],
                    in0=ps, in1=bkv_sb[:, 0:1].to_broadcast((P, 512)),
                    op=mybir.AluOpType.add)

            def proj_phase_b0(xt_t, qt_sb, kvt_sb):
                """Paced projection for batch 0: matmuls chase the xt DMAs.
                Round 1 (DMA-paced): qc0 both halves, qc1 sh0, kv sh0.
                Round 2 (chunks resident): qc1 sh1, kv sh1."""
                a0 = psum.tile([P, 1024], F32, tag="A", bufs=2)
                a1 = psum.tile([P, 1024], F32, tag="A", bufs=2)
                a2 = psum.tile([P, 1024], F32, tag="av", bufs=1)
                c0 = psum.tile([P, 512], F32, tag="C", bufs=2)
                c1 = psum.tile([P, 512], F32, tag="C", bufs=2)
                for hc in range(HC):
                    st, sp = hc == 0, hc == HC - 1
                    for qc, sh, ps in ((0, 0, a0), (0, 1, a1), (1, 0, a2)):
                        for j in range(2):
                            nc.tensor.matmul(
                                ps[:, j * 512:(j + 1) * 512],
                                lhsT=wqc[hc][:, qc * P:(qc + 1) * P],
                                rhs=xt_t[hc][:, sh * 1024 + j * 512:
                                             sh * 1024 + (j + 1) * 512],
                                start=st, stop=sp, skip_group_check=True)
                    for j, ps in ((0, c0), (1, c1)):
                        nc.tensor.matmul(
                            ps,
                            lhsT=wkvc[hc],
                            rhs=xt_t[hc][:, j * 512:(j + 1) * 512],
                            start=st, stop=sp, skip_group_check=True)
                proj_drain_qt(qt_sb, a0, 0, 0)
                proj_drain_qt(qt_sb, a1, 0, 1)
                proj_drain_qt(qt_sb, a2, 1, 0)
                proj_drain_kv(kvt_sb, c0, 0, 0)
                proj_drain_kv(kvt_sb, c1, 0, 1)
                a3 = psum.tile([P, 1024], F32, tag="A", bufs=2)
                c2 = psum.tile([P, 512], F32, tag="C", bufs=2)
                c3 = psum.tile([P, 512], F32, tag="C", bufs=2)
                for hc in range(HC):
                    st, sp = hc == 0, hc == HC - 1
                    for j in range(2):
                        nc.tensor.matmul(
                            a3[:, j * 512:(j + 1) * 512],
                            lhsT=wqc[hc][:, P:2 * P],
                            rhs=xt_t[hc][:, 1024 + j * 512:
                                         1024 + (j + 1) * 512],
                            start=st, stop=sp, skip_group_check=True)
                    for j, ps in ((0, c2), (1, c3)):
                        nc.tensor.matmul(
                            ps,
                            lhsT=wkvc[hc],
                            rhs=xt_t[hc][:, 1024 + j * 512:
                                         1024 + (j + 1) * 512],
                            start=st, stop=sp, skip_group_check=True)
                proj_drain_qt(qt_sb, a3, 1, 1)
                proj_drain_kv(kvt_sb, c2, 1, 0)
                proj_drain_kv(kvt_sb, c3, 1, 1)
                nc.sync.dma_start(bo_sb, bo_d)

            def make_proj_thunks(b, xt_t, qt_sb, kvt_sb, group):
                """Deferred projections for batch b, emitted as fillers.
                group: "kv" | "qc0" | "qc1" — kv first so K/V finish early."""
                if group == "kv":
                    chains = [("kv", None, sh, j) for sh in range(2)
                              for j in range(2)]
                else:
                    qc = int(group[2])
                    chains = [("qt", qc, sh, j) for sh in range(2)
                              for j in range(2)]

                def chain_thunks(kind, qc, sh, j):
                    ps_box = {}

                    def start_thunk():
                        ps_box["ps"] = psum.tile([P, 512], F32, tag="C", bufs=2,
                                                 name="projps")
                        return 0

                    def mk_mm(t):
                        def mm_thunk():
                            ps = ps_box["ps"]
                            for hc in (2 * t, 2 * t + 1):
                                st, sp = hc == 0, hc == HC - 1
                                if kind == "qt":
                                    nc.tensor.matmul(
                                        ps,
                                        lhsT=wqc[hc][:, qc * P:(qc + 1) * P],
                                        rhs=xt_t[hc][:, sh * 1024 + j * 512:
                                                     sh * 1024 + (j + 1) * 512],
                                        start=st, stop=sp,
                                        skip_group_check=True)
                                else:
                                    nc.tensor.matmul(
                                        ps,
                                        lhsT=wkvc[hc],
                                        rhs=xt_t[hc][:, sh * 1024 + j * 512:
                                                     sh * 1024 + (j + 1) * 512],
                                        start=st, stop=sp,
                                        skip_group_check=True)
                            return 2
                        return mm_thunk

                    def drain_thunk():
                        ps = ps_box["ps"]
                        if kind == "qt":
                            nc.vector.tensor_tensor(
                                out=qt_sb[:, qc, sh * 1024 + j * 512:
                                          sh * 1024 + (j + 1) * 512],
                                in0=ps,
                                in1=bq_sb[:, qc:qc + 1].to_broadcast((P, 512)),
                                op=mybir.AluOpType.add)
                        else:
                            proj_drain_kv(kvt_sb, ps, sh, j)
                        return 0

                    yield start_thunk
                    for t in range(HC // 2):
                        yield mk_mm(t)
                    yield drain_thunk

                for ch in chains:
                    for th in chain_thunks(*ch):
                        work.append(th)

            def finish_kv(b, kvt_sb):
                """K replica at partitions 64-127 + V transpose via XBAR."""
                kt2_sb = kvt_pool.tile([P, S], BF16, tag="kt2")
                nc.sync.dma_start(kt2_sb[64:128, :], kvt_sb[0:64, :])
                # XBAR transpose needs a contiguous destination; land V there
                # and splice the softmax-denominator ones column on the DVE.
                vpt = vp_pool.tile([P, SC, D], BF16, tag="vpt")
                nc.sync.dma_start_transpose(vpt, kvt_sb[64:128, :])
                vp_sb = vp_pool.tile([P, SC, 65], BF16, tag="vp")
                nc.vector.memset(vp_sb[:, :, 64], 1.0)
                nc.vector.tensor_copy(out=vp_sb[:, :, 0:64], in_=vpt)
                return kt2_sb, vp_sb

            # ---------------- out-projection chains ----------------
            # wo column-slices are loaded exactly once: jq0/jq1 into the wos
            # pool, jq2/jq3 into the xt chunk buffers (dead after batch-1
            # projections). wo_rhs resolves (jq, sq) -> AP at emission time.
            wo_parts = {}

            def wo_rhs(jq, sq):
                if jq < 2:
                    return wo_parts[jq][:, sq, :]
                t = wo_parts[(jq, sq // 8)]
                return t[:, (sq % 8) // 4,
                         (sq % 4) * 512:(sq % 4 + 1) * 512]

            def outproj_load(jq):
                def load_thunk(jq=jq):
                    if jq < 2:
                        w = wos_pool.tile([P, SC, 512], BF16, tag="wo",
                                          name="wosl")
                        nc.sync.dma_start(w, wo_d[:, jq])
                        wo_parts[jq] = w
                    else:
                        for q in range(2):
                            t = xt_pool.tile(
                                [P, 2, S], BF16,
                                tag=f"xt{(jq - 2) * 2 + q + 4}",
                                name="woxt")
                            nc.sync.dma_start(
                                t, wo_d[:, jq, 8 * q:8 * q + 8, :]
                                .rearrange("p (c x) j -> p c (x j)", c=2))
                            wo_parts[(jq, q)] = t
                    return 0
                work.append(load_thunk)

            def outproj_chain(b, half, rc, jq):
                """Thunks: one [128,512] row-chunk x col-slice chain."""
                box = {}

                def start_thunk():
                    box["ps"] = psum.tile([P, 512], F32, tag="C",
                                          bufs=2, name="outps")
                    return 0
                work.append(start_thunk)

                def mk_mm(t):
                    def mm_thunk():
                        for sq in (2 * t, 2 * t + 1):
                            nc.tensor.matmul(
                                box["ps"],
                                lhsT=half[:, sq, :],
                                rhs=wo_rhs(jq, sq),
                                start=sq == 0, stop=sq == SC - 1,
                                skip_group_check=True)
                        return 2
                    return mm_thunk
                for t in range(SC // 2):
                    work.append(mk_mm(t))

                def drain_thunk():
                    o = out_pool.tile([P, 512], F32, tag="out")
                    nc.vector.tensor_tensor(
                        out=o, in0=box["ps"],
                        in1=bo_sb[:, jq * 512:(jq + 1) * 512],
                        op=mybir.AluOpType.add)
                    nc.sync.dma_start(
                        out_d[b, rc * P:(rc + 1) * P,
                              jq * 512:(jq + 1) * 512], o)
                    return 0
                work.append(drain_thunk)

            # ---------------- attention ----------------
            def attn_phase(b, qt_sb, kvt_sb, kt2_sb, vp_sb, attn_halves,
                           gate_hook=None):
                pair_state = {}
                for h in range(4):
                    pbase = (h % 2) * 64
                    qt_h = qt_sb[pbase:pbase + 64, h // 2, :]
                    kt_h = (kvt_sb if h % 2 == 0 else kt2_sb)[
                        pbase:pbase + 64, :]
                    attnT_sb = attnT_pool.tile([65, S], BF16, tag="attnT")
                    araw = araw_pool.tile([P, SC, D], BF16,
                                          tag=f"araw{h % 2}")
                    if h % 2 == 0:
                        pair_state.clear()
                        den = dxt_pool.tile([16, S], BF16, tag="den")
                        nc.vector.memset(den, 1.0)
                        rcp0 = rcp_pool.tile([P, SC], F32, tag="rcp")
                        rcp1 = rcp_pool.tile([P, SC], F32, tag="rcp")
                        pair_state.update(den=den, araw0=araw,
                                          rcp0=rcp0, rcp1=rcp1)
                    else:
                        den = pair_state["den"]
                        pair_state["araw1"] = araw
                    for qtp in range(2):
                        av = psum.tile([P, 1024], F32, tag="av", bufs=1)
                        prev = None
                        for kc in range(SC):
                            sc_ps = psum.tile([P, 1024], F32, tag="A", bufs=2)
                            for j in range(2):
                                nc.tensor.matmul(
                                    sc_ps[:, j * 512:(j + 1) * 512],
                                    lhsT=kt_h[:, kc * P:(kc + 1) * P],
                                    rhs=qt_h[:, qtp * 1024 + j * 512:
                                             qtp * 1024 + (j + 1) * 512],
                                    start=True, stop=True)
                            if prev is not None:
                                ppt, pkc = prev
                                for j in range(2):
                                    nc.tensor.matmul(
                                        av[0:65, j * 512:(j + 1) * 512],
                                        lhsT=vp_sb[:, pkc, :],
                                        rhs=ppt[:, j * 512:(j + 1) * 512],
                                        start=(pkc == 0), stop=(pkc == SC - 1),
                                        skip_group_check=True)
                            if kc % 2 == 1 and not (b == 0 and h == 0
                                                    and qtp == 0):
                                pump(2 if (b == 1 or h >= 2) else 1)
                            pt = pt_pool.tile([P, 1024], BF16, tag="pt")
                            nc.scalar.activation(
                                pt, sc_ps, mybir.ActivationFunctionType.Exp)
                            prev = (pt, kc)
                        ppt, pkc = prev
                        for j in range(2):
                            nc.tensor.matmul(
                                av[0:65, j * 512:(j + 1) * 512],
                                lhsT=vp_sb[:, pkc, :],
                                rhs=ppt[:, j * 512:(j + 1) * 512],
                                start=(pkc == 0), stop=(pkc == SC - 1),
                                skip_group_check=True)
                        nc.scalar.copy(
                            attnT_sb[:, qtp * 1024:(qtp + 1) * 1024],
                            av[0:65, :])
                        # incremental transpose + normalize per q-half, all
                        # off the PE: attn rows via XBAR; the softmax
                        # denominator row (64) via a 16-partition-aligned
                        # collect tile (XBAR offset rule), one per head-pair.
                        hw = slice(qtp * 1024, (qtp + 1) * 1024)
                        cw = slice(qtp * 8, (qtp + 1) * 8)
                        nc.sync.dma_start_transpose(
                            araw[:, cw, :], attnT_sb[0:64, hw])
                        nc.sync.dma_start(den[h % 2:h % 2 + 1, hw],
                                          attnT_sb[64:65, hw])
                        if h % 2 == 1:
                            dxt = dxt_pool.tile([P, 8, 16], BF16, tag="dxt")
                            nc.sync.dma_start_transpose(dxt, den[:, hw])
                            half = attn_halves[h // 2]
                            for hh in range(2):
                                rcp = pair_state[f"rcp{hh}"]
                                nc.vector.reciprocal(rcp[:, cw],
                                                     dxt[:, :, hh])
                                ar = pair_state[f"araw{hh}"]
                                eng = nc.vector if hh == 0 else nc.gpsimd
                                for tb in range(qtp * 8, qtp * 8 + 8):
                                    eng.tensor_tensor(
                                        out=half[:, tb,
                                                 hh * D:(hh + 1) * D],
                                        in0=ar[:, tb, :],
                                        in1=rcp[:, tb:tb + 1]
                                        .to_broadcast((P, D)),
                                        op=mybir.AluOpType.mult)
                        pump(4 if b == 1 else 3)
                    if gate_hook is not None:
                        gate_hook(h)

            # ================= schedule =================
            # batch 0 projections, paced against the xt chunk DMAs
            xt0 = load_xt(0)
            qt0 = qt_pool.tile([P, 2, S], BF16, tag="qt")
            kvt0 = kvt_pool.tile([P, S], BF16, tag="kvt")
            proj_phase_b0(xt0, qt0, kvt0)
            kt20, vp0 = finish_kv(0, kvt0)

            # batch 1 xt load starts as soon as batch 0 mms release chunks
            xt1 = load_xt(1)
            qt1 = qt_pool.tile([P, 2, S], BF16, tag="qt")
            kvt1 = kvt_pool.tile([P, S], BF16, tag="kvt")
            kv1_box = {}

            def finish_kv1_thunk():
                kv1_box["r"] = finish_kv(1, kvt1)
                return 0
            make_proj_thunks(1, xt1, qt1, kvt1, "kv")
            work.append(finish_kv1_thunk)
            make_proj_thunks(1, xt1, qt1, kvt1, "qc0")
            make_proj_thunks(1, xt1, qt1, kvt1, "qc1")
            # wo jq2/jq3 slices into the freed xt chunk buffers
            outproj_load(2)
            outproj_load(3)

            attn0 = [attn_pool.tile([P, SC, P], BF16, tag=f"attn{i}",
                                     name=f"attn0_{i}") for i in range(2)]
            attn1 = [attn_pool.tile([P, SC, P], BF16, tag=f"attn{i}",
                                     name=f"attn1_{i}") for i in range(2)]

            # batch 0 attention, consuming batch 1 projection fillers
            attn_phase(0, qt0, kvt0, kt20, vp0, attn0)
            flush()
            kt21, vp1 = kv1_box["r"]

            # batch 1 attention, consuming out-projection fillers:
            # batch 0 rows first; batch-1 heads 0/1 rows as soon as ready
            outproj_load(0)
            outproj_load(1)
            for jq in range(4):
                outproj_chain(0, attn0[0], 0, jq)
                outproj_chain(0, attn0[1], 1, jq)

            def gate(h):
                if h == 1:
                    for jq in range(4):
                        outproj_chain(1, attn1[0], 0, jq)

            attn_phase(1, qt1, kvt1, kt21, vp1, attn1, gate_hook=gate)
            for jq in range(4):
                outproj_chain(1, attn1[1], 1, jq)
            flush()

    nc.compile()
    return nc


def _get_nc():
    if "nc" not in _CACHE:
        _CACHE["nc"] = _build()
    return _CACHE["nc"]


def _prep_inputs(hidden_state, w_q, b_q, w_k, b_k, w_v, b_v, w_o, b_o):
    """Host-side sharding/layout prep. Only layout/dtype transforms."""
    xt = np.ascontiguousarray(hidden_state.transpose(0, 2, 1)).astype(NP_BF16)
    # wo pre-arranged to [P, 4, SC, 512] so each column-slice DMA line is
    # one contiguous 16KB run per partition
    wo = np.ascontiguousarray(
        w_o.reshape(SC, P, 4, 512).transpose(1, 2, 0, 3)).astype(NP_BF16)
    bo = np.broadcast_to(b_o.astype(np.float32), (P, HID)).copy()
    in_maps = []
    for g in range(NCORES):
        wq_g = np.ascontiguousarray(
            (w_q[:, g * QF:(g + 1) * QF] * 0.125)
            .reshape(HC, P, QF).transpose(1, 0, 2)).astype(NP_BF16)
        bq_g = np.ascontiguousarray(
            (b_q[g * QF:(g + 1) * QF] * 0.125).reshape(2, P)).astype(np.float32)
        wkv_g = np.ascontiguousarray(np.concatenate(
            [w_k[:, g * D:(g + 1) * D], w_v[:, g * D:(g + 1) * D]],
            axis=1).reshape(HC, P, P).transpose(1, 0, 2)).astype(NP_BF16)
        bkv_g = np.ascontiguousarray(np.concatenate(
            [b_k[g * D:(g + 1) * D], b_v[g * D:(g + 1) * D]])
            .reshape(P, 1)).astype(np.float32)
        in_maps.append({
            "xt": xt, "wq": wq_g, "bq": bq_g, "wkv": wkv_g, "bkv": bkv_g,
            "wo": wo, "bo": bo,
        })
    return in_maps


def kernel(hidden_state, w_q, b_q, w_k, b_k, w_v, b_v, w_o, b_o,
           _trace=False):
    hidden_state = np.asarray(hidden_state, np.float32)
    args = [np.asarray(a, np.float32) for a in
            (w_q, b_q, w_k, b_k, w_v, b_v, w_o, b_o)]
    nc = _get_nc()
    in_maps = _prep_inputs(hidden_state, *args)
    res = bass_utils.run_bass_kernel_spmd(
        nc, in_maps, core_ids=list(range(NCORES)), trace=_trace)
    out = np.concatenate([res.results[g]["out"] for g in range(NCORES)],
                         axis=1).astype(np.float32)
    if _trace:
        _CACHE["last_results"] = res
    return out


# revision 29
# speedup vs baseline: 1.0166x; 1.0110x over previous
"""GQA kernel for Trainium2, sharded over 8 NeuronCores.

Sharding: tensor-parallel over heads. Core g owns Q heads 4g..4g+3 and KV
group g (GQA rep=4, so all 4 local heads share one K/V). The reference's
final projection contracts over the *sequence* axis (faithful swapaxes
quirk), so output rows partition cleanly by head: core g produces rows
g*256..(g+1)*256 of the [2, 2048, 2048] output. No collectives.

v2 schedule (software-pipelined, trace-driven):
  - xt is loaded per-hidden-chunk; projection matmuls start as chunks land.
  - attention inner loop is ACT(exp)-bound; batch-1 projection and batch-0
    out-projection matmul chains are injected between attention matmuls as
    "filler" thunks so the PE never idles while ACT chews exp tiles.
  - all transposes (V, attn, softmax denominator) run on the DMA XBAR
    (dma_start_transpose), not the PE.
  - out-projection is split per head-pair so rows for heads 0/1 of batch 1
    are projected while heads 2/3 still run attention; only the last
    quarter of the out-projection remains after the attention pipeline.
"""
import numpy as np
import ml_dtypes
from collections import deque

import concourse.bass as bass
import concourse.bacc as bacc
import concourse.mybir as mybir
import concourse.tile as tile
from concourse import bass_utils

BF16 = mybir.dt.bfloat16
F32 = mybir.dt.float32
NP_BF16 = ml_dtypes.bfloat16

B, S, HID = 2, 2048, 2048
NCORES = 8
HEADS_PER_CORE = 4   # of 32
D = 64               # head dim
QF = HEADS_PER_CORE * D   # 256 q-features per core
P = 128
HC = HID // P        # 16 hidden chunks
SC = S // P          # 16 seq chunks

_CACHE = {}


def _build():
    nc = bacc.Bacc("TRN2", target_bir_lowering=False, debug=False,
                   num_devices=NCORES)
    # ---- DRAM I/O ----
    xt_d = nc.dram_tensor("xt", [B, HID, S], BF16, kind="ExternalInput").ap()
    # weights arrive pre-arranged on host so every DMA line is contiguous
    wq_d = nc.dram_tensor("wq", [P, HC, QF], BF16, kind="ExternalInput").ap()
    bq_d = nc.dram_tensor("bq", [2, P], F32, kind="ExternalInput").ap()
    wkv_d = nc.dram_tensor("wkv", [P, HC, P], BF16, kind="ExternalInput").ap()
    bkv_d = nc.dram_tensor("bkv", [P, 1], F32, kind="ExternalInput").ap()
    wo_d = nc.dram_tensor("wo", [P, 4, SC, 512], BF16,
                          kind="ExternalInput").ap()
    bo_d = nc.dram_tensor("bo", [P, HID], F32, kind="ExternalInput").ap()
    out_d = nc.dram_tensor("out", [B, QF, HID], F32, kind="ExternalOutput").ap()

    with tile.TileContext(nc) as tc:
        with (
            tc.tile_pool(name="consts", bufs=1) as consts,
            tc.tile_pool(name="xt", bufs=1) as xt_pool,
            tc.tile_pool(name="qt", bufs=2) as qt_pool,
            tc.tile_pool(name="kvt", bufs=2) as kvt_pool,
            tc.tile_pool(name="vp", bufs=2) as vp_pool,
            tc.tile_pool(name="pt", bufs=2) as pt_pool,
            tc.tile_pool(name="attnT", bufs=2) as attnT_pool,
            tc.tile_pool(name="araw", bufs=2) as araw_pool,
            tc.tile_pool(name="dxt", bufs=2) as dxt_pool,
            tc.tile_pool(name="rcp", bufs=2) as rcp_pool,
            tc.tile_pool(name="attn", bufs=2) as attn_pool,
            tc.tile_pool(name="wos", bufs=2) as wos_pool,
            tc.tile_pool(name="outp", bufs=2) as out_pool,
            tc.tile_pool(name="psum", bufs=1, space="PSUM") as psum,
        ):
            # ---- constants (bo deferred: not needed until out-proj) ----
            bq_sb = consts.tile([P, 2], F32)
            nc.sync.dma_start(bq_sb, bq_d.rearrange("c p -> p c"))
            bkv_sb = consts.tile([P, 1], F32)
            nc.sync.dma_start(bkv_sb, bkv_d)
            wq_sb = consts.tile([P, HC, QF], BF16)
            nc.sync.dma_start(wq_sb, wq_d)
            wkv_sb = consts.tile([P, HC, P], BF16)
            nc.sync.dma_start(wkv_sb, wkv_d)
            wqc = [wq_sb[:, hc] for hc in range(HC)]
            wkvc = [wkv_sb[:, hc] for hc in range(HC)]
            bo_sb = consts.tile([P, HID], F32)

            # filler thunk queue: each thunk emits a small chunk of deferred
            # PE work (or a DMA, cost 0) and returns its PE cost in matmuls.
            work = deque()

            def pump(budget=1):
                spent = 0
                while work and spent < budget:
                    spent += work.popleft()()

            def flush():
                while work:
                    work.popleft()()

            # ---------------- projections ----------------
            def load_xt(b):
                """xt in chunk-pairs: fewer triggers, 8KB DMA lines."""
                pairs = []
                for hp in range(HC // 2):
                    t = xt_pool.tile([P, 2, S], BF16, tag=f"xt{hp}",
                                     name=f"xtc{hp}")
                    nc.sync.dma_start(
                        t, xt_d[b, hp * 2 * P:(hp + 1) * 2 * P, :]
                        .rearrange("(c p) s -> p c s", p=P))
                    pairs.append(t)
                return [pairs[hc // 2][:, hc % 2, :] for hc in range(HC)]

            def proj_drain_qt(qt_sb, ps, qc, sh):
                nc.vector.tensor_tensor(
                    out=qt_sb[:, qc, sh * 1024:(sh + 1) * 1024],
                    in0=ps, in1=bq_sb[:, qc:qc + 1].to_broadcast((P, 1024)),
                    op=mybir.AluOpType.add)

            def proj_drain_kv(kvt_sb, ps, sh, j):
                nc.vector.tensor_tensor(
                    out=kvt_sb[:, sh * 1024 + j * 512:sh * 1024 + (j + 1) * 512],
                    in0=ps, in1=bkv_sb[:, 0:1].to_broadcast((P, 512)),
                    op=mybir.AluOpType.add)

            def proj_phase_b0(xt_t, qt_sb, kvt_sb):
                """DMA-paced projection round for batch 0: qc0 (heads 0/1)
                and the full K/V — everything attention needs to start.
                qc1 (heads 2/3) is deferred to filler thunks."""
                a0 = psum.tile([P, 1024], F32, tag="A", bufs=2)
                a1 = psum.tile([P, 1024], F32, tag="A", bufs=2)
                a2 = psum.tile([P, 1024], F32, tag="av", bufs=1)
                c0 = psum.tile([P, 512], F32, tag="C", bufs=2)
                c1 = psum.tile([P, 512], F32, tag="C", bufs=2)
                for hc in range(HC):
                    st, sp = hc == 0, hc == HC - 1
                    for sh, ps in ((0, a0), (1, a1)):
                        for j in range(2):
                            nc.tensor.matmul(
                                ps[:, j * 512:(j + 1) * 512],
                                lhsT=wqc[hc][:, 0:P],
                                rhs=xt_t[hc][:, sh * 1024 + j * 512:
                                             sh * 1024 + (j + 1) * 512],
                                start=st, stop=sp, skip_group_check=True)
                    for j in range(2):
                        nc.tensor.matmul(
                            a2[:, j * 512:(j + 1) * 512],
                            lhsT=wkvc[hc],
                            rhs=xt_t[hc][:, 1024 + j * 512:
                                         1024 + (j + 1) * 512],
                            start=st, stop=sp, skip_group_check=True)
                    for j, ps in ((0, c0), (1, c1)):
                        nc.tensor.matmul(
                            ps,
                            lhsT=wkvc[hc],
                            rhs=xt_t[hc][:, j * 512:(j + 1) * 512],
                            start=st, stop=sp, skip_group_check=True)
                proj_drain_qt(qt_sb, a0, 0, 0)
                proj_drain_qt(qt_sb, a1, 0, 1)
                proj_drain_kv(kvt_sb, c0, 0, 0)
                proj_drain_kv(kvt_sb, c1, 0, 1)
                nc.vector.tensor_tensor(
                    out=kvt_sb[:, 1024:2048], in0=a2,
                    in1=bkv_sb[:, 0:1].to_broadcast((P, 1024)),
                    op=mybir.AluOpType.add)
                nc.sync.dma_start(bo_sb, bo_d)

            def make_proj_thunks(b, xt_t, qt_sb, kvt_sb, group):
                """Deferred projections for batch b, emitted as fillers.
                group: "kv" | "qc0" | "qc1" — kv first so K/V finish early."""
                if group == "kv":
                    chains = [("kv", None, sh, j) for sh in range(2)
                              for j in range(2)]
                else:
                    qc = int(group[2])
                    chains = [("qt", qc, sh, j) for sh in range(2)
                              for j in range(2)]

                def chain_thunks(kind, qc, sh, j):
                    ps_box = {}

                    def start_thunk():
                        ps_box["ps"] = psum.tile([P, 512], F32, tag="C", bufs=2,
                                                 name="projps")
                        return 0

                    def mk_mm(t):
                        def mm_thunk():
                            ps = ps_box["ps"]
                            for hc in (2 * t, 2 * t + 1):
                                st, sp = hc == 0, hc == HC - 1
                                if kind == "qt":
                                    nc.tensor.matmul(
                                        ps,
                                        lhsT=wqc[hc][:, qc * P:(qc + 1) * P],
                                        rhs=xt_t[hc][:, sh * 1024 + j * 512:
                                                     sh * 1024 + (j + 1) * 512],
                                        start=st, stop=sp,
                                        skip_group_check=True)
                                else:
                                    nc.tensor.matmul(
                                        ps,
                                        lhsT=wkvc[hc],
                                        rhs=xt_t[hc][:, sh * 1024 + j * 512:
                                                     sh * 1024 + (j + 1) * 512],
                                        start=st, stop=sp,
                                        skip_group_check=True)
                            return 2
                        return mm_thunk

                    def drain_thunk():
                        ps = ps_box["ps"]
                        if kind == "qt":
                            nc.vector.tensor_tensor(
                                out=qt_sb[:, qc, sh * 1024 + j * 512:
                                          sh * 1024 + (j + 1) * 512],
                                in0=ps,
                                in1=bq_sb[:, qc:qc + 1].to_broadcast((P, 512)),
                                op=mybir.AluOpType.add)
                        else:
                            proj_drain_kv(kvt_sb, ps, sh, j)
                        return 0

                    yield start_thunk
                    for t in range(HC // 2):
                        yield mk_mm(t)
                    yield drain_thunk

                for ch in chains:
                    for th in chain_thunks(*ch):
                        work.append(th)

            def finish_kv(b, kvt_sb):
                """K replica at partitions 64-127 + V transpose via XBAR."""
                kt2_sb = kvt_pool.tile([P, S], BF16, tag="kt2")
                nc.sync.dma_start(kt2_sb[64:128, :], kvt_sb[0:64, :])
                # XBAR transpose needs a contiguous destination; land V there
                # and splice the softmax-denominator ones column on the DVE.
                vpt = vp_pool.tile([P, SC, D], BF16, tag="vpt")
                nc.sync.dma_start_transpose(vpt, kvt_sb[64:128, :])
                vp_sb = vp_pool.tile([P, SC, 65], BF16, tag="vp")
                nc.vector.memset(vp_sb[:, :, 64], 1.0)
                nc.vector.tensor_copy(out=vp_sb[:, :, 0:64], in_=vpt)
                return kt2_sb, vp_sb

            # ---------------- out-projection chains ----------------
            # wo column-slices are loaded exactly once: jq0/jq1 into the wos
            # pool, jq2/jq3 into the xt chunk buffers (dead after batch-1
            # projections). wo_rhs resolves (jq, sq) -> AP at emission time.
            wo_parts = {}

            def wo_rhs(jq, sq):
                if jq < 2:
                    return wo_parts[jq][:, sq, :]
                t = wo_parts[(jq, sq // 8)]
                return t[:, (sq % 8) // 4,
                         (sq % 4) * 512:(sq % 4 + 1) * 512]

            def outproj_load(jq):
                def load_thunk(jq=jq):
                    if jq < 2:
                        w = wos_pool.tile([P, SC, 512], BF16, tag="wo",
                                          name="wosl")
                        nc.sync.dma_start(w, wo_d[:, jq])
                        wo_parts[jq] = w
                    else:
                        for q in range(2):
                            t = xt_pool.tile(
                                [P, 2, S], BF16,
                                tag=f"xt{(jq - 2) * 2 + q + 4}",
                                name="woxt")
                            nc.sync.dma_start(
                                t, wo_d[:, jq, 8 * q:8 * q + 8, :]
                                .rearrange("p (c x) j -> p c (x j)", c=2))
                            wo_parts[(jq, q)] = t
                    return 0
                work.append(load_thunk)

            def outproj_chain(b, half, rc, jq):
                """Thunks: one [128,512] row-chunk x col-slice chain."""
                box = {}

                def start_thunk():
                    box["ps"] = psum.tile([P, 512], F32, tag="C",
                                          bufs=2, name="outps")
                    return 0
                work.append(start_thunk)

                def mk_mm(t):
                    def mm_thunk():
                        for sq in (2 * t, 2 * t + 1):
                            nc.tensor.matmul(
                                box["ps"],
                                lhsT=half[:, sq, :],
                                rhs=wo_rhs(jq, sq),
                                start=sq == 0, stop=sq == SC - 1,
                                skip_group_check=True)
                        return 2
                    return mm_thunk
                for t in range(SC // 2):
                    work.append(mk_mm(t))

                def drain_thunk():
                    o = out_pool.tile([P, 512], F32, tag="out")
                    nc.vector.tensor_tensor(
                        out=o, in0=box["ps"],
                        in1=bo_sb[:, jq * 512:(jq + 1) * 512],
                        op=mybir.AluOpType.add)
                    nc.sync.dma_start(
                        out_d[b, rc * P:(rc + 1) * P,
                              jq * 512:(jq + 1) * 512], o)
                    return 0
                work.append(drain_thunk)

            # ---------------- attention ----------------
            def attn_phase(b, qt_sb, kvt_sb, kt2_sb, vp_sb, attn_halves,
                           gate_hook=None):
                pair_state = {}
                for h in range(4):
                    pbase = (h % 2) * 64
                    qt_h = qt_sb[pbase:pbase + 64, h // 2, :]
                    kt_h = (kvt_sb if h % 2 == 0 else kt2_sb)[
                        pbase:pbase + 64, :]
                    attnT_sb = attnT_pool.tile([65, S], BF16, tag="attnT")
                    araw = araw_pool.tile([P, SC, D], BF16,
                                          tag=f"araw{h % 2}")
                    if h % 2 == 0:
                        pair_state.clear()
                        den = dxt_pool.tile([16, S], BF16, tag="den")
                        nc.vector.memset(den, 1.0)
                        rcp0 = rcp_pool.tile([P, SC], F32, tag="rcp")
                        rcp1 = rcp_pool.tile([P, SC], F32, tag="rcp")
                        pair_state.update(den=den, araw0=araw,
                                          rcp0=rcp0, rcp1=rcp1)
                    else:
                        den = pair_state["den"]
                        pair_state["araw1"] = araw
                    for qtp in range(2):
                        av = psum.tile([P, 1024], F32, tag="av", bufs=1)
                        prev = None
                        for kc in range(SC):
                            sc_ps = psum.tile([P, 1024], F32, tag="A", bufs=2)
                            for j in range(2):
                                nc.tensor.matmul(
                                    sc_ps[:, j * 512:(j + 1) * 512],
                                    lhsT=kt_h[:, kc * P:(kc + 1) * P],
                                    rhs=qt_h[:, qtp * 1024 + j * 512:
                                             qtp * 1024 + (j + 1) * 512],
                                    start=True, stop=True)
                            if prev is not None:
                                ppt, pkc = prev
                                for j in range(2):
                                    nc.tensor.matmul(
                                        av[0:65, j * 512:(j + 1) * 512],
                                        lhsT=vp_sb[:, pkc, :],
                                        rhs=ppt[:, j * 512:(j + 1) * 512],
                                        start=(pkc == 0), stop=(pkc == SC - 1),
                                        skip_group_check=True)
                            if kc % 2 == 1 and not (b == 0 and h == 0
                                                    and qtp == 0):
                                pump(2 if (b == 1 or h >= 2) else 1)
                            pt = pt_pool.tile([P, 1024], BF16, tag="pt")
                            nc.scalar.activation(
                                pt, sc_ps, mybir.ActivationFunctionType.Exp)
                            prev = (pt, kc)
                        ppt, pkc = prev
                        for j in range(2):
                            nc.tensor.matmul(
                                av[0:65, j * 512:(j + 1) * 512],
                                lhsT=vp_sb[:, pkc, :],
                                rhs=ppt[:, j * 512:(j + 1) * 512],
                                start=(pkc == 0), stop=(pkc == SC - 1),
                                skip_group_check=True)
                        nc.scalar.copy(
                            attnT_sb[:, qtp * 1024:(qtp + 1) * 1024],
                            av[0:65, :])
                        # incremental transpose + normalize per q-half, all
                        # off the PE: attn rows via XBAR; the softmax
                        # denominator row (64) via a 16-partition-aligned
                        # collect tile (XBAR offset rule), one per head-pair.
                        hw = slice(qtp * 1024, (qtp + 1) * 1024)
                        cw = slice(qtp * 8, (qtp + 1) * 8)
                        nc.sync.dma_start_transpose(
                            araw[:, cw, :], attnT_sb[0:64, hw])
                        nc.sync.dma_start(den[h % 2:h % 2 + 1, hw],
                                          attnT_sb[64:65, hw])
                        if h % 2 == 1:
                            dxt = dxt_pool.tile([P, 8, 16], BF16, tag="dxt")
                            nc.sync.dma_start_transpose(dxt, den[:, hw])
                            half = attn_halves[h // 2]
                            for hh in range(2):
                                rcp = pair_state[f"rcp{hh}"]
                                nc.vector.reciprocal(rcp[:, cw],
                                                     dxt[:, :, hh])
                                ar = pair_state[f"araw{hh}"]
                                eng = nc.vector if hh == 0 else nc.gpsimd
                                for tb in range(qtp * 8, qtp * 8 + 8):
                                    eng.tensor_tensor(
                                        out=half[:, tb,
                                                 hh * D:(hh + 1) * D],
                                        in0=ar[:, tb, :],
                                        in1=rcp[:, tb:tb + 1]
                                        .to_broadcast((P, D)),
                                        op=mybir.AluOpType.mult)
                        pump(4 if b == 1 else 3)
                    if gate_hook is not None:
                        gate_hook(h)

            # ================= schedule =================
            # batch 0 projections, paced against the xt chunk DMAs
            xt0 = load_xt(0)
            qt0 = qt_pool.tile([P, 2, S], BF16, tag="qt")
            kvt0 = kvt_pool.tile([P, S], BF16, tag="kvt")
            proj_phase_b0(xt0, qt0, kvt0)
            kt20, vp0 = finish_kv(0, kvt0)
            # heads 2/3 Q projection rides as the first fillers
            make_proj_thunks(0, xt0, qt0, kvt0, "qc1")

            # batch 1 xt load starts as soon as batch 0 mms release chunks
            xt1 = load_xt(1)
            qt1 = qt_pool.tile([P, 2, S], BF16, tag="qt")
            kvt1 = kvt_pool.tile([P, S], BF16, tag="kvt")
            kv1_box = {}

            def finish_kv1_thunk():
                kv1_box["r"] = finish_kv(1, kvt1)
                return 0
            make_proj_thunks(1, xt1, qt1, kvt1, "kv")
            work.append(finish_kv1_thunk)
            make_proj_thunks(1, xt1, qt1, kvt1, "qc0")
            make_proj_thunks(1, xt1, qt1, kvt1, "qc1")
            # wo jq2/jq3 slices into the freed xt chunk buffers
            outproj_load(2)
            outproj_load(3)

            attn0 = [attn_pool.tile([P, SC, P], BF16, tag=f"attn{i}",
                                     name=f"attn0_{i}") for i in range(2)]
            attn1 = [attn_pool.tile([P, SC, P], BF16, tag=f"attn{i}",
                                     name=f"attn1_{i}") for i in range(2)]

            # batch 0 attention, consuming batch 1 projection fillers
            attn_phase(0, qt0, kvt0, kt20, vp0, attn0)
            flush()
            kt21, vp1 = kv1_box["r"]

            # batch 1 attention, consuming out-projection fillers:
            # batch 0 rows first; batch-1 heads 0/1 rows as soon as ready
            outproj_load(0)
            outproj_load(1)
            for jq in range(4):
                outproj_chain(0, attn0[0], 0, jq)
                outproj_chain(0, attn0[1], 1, jq)

            def gate(h):
                if h == 1:
                    for jq in range(4):
                        outproj_chain(1, attn1[0], 0, jq)

            attn_phase(1, qt1, kvt1, kt21, vp1, attn1, gate_hook=gate)
            for jq in range(4):
                outproj_chain(1, attn1[1], 1, jq)
            flush()

    nc.compile()
    return nc


def _get_nc():
    if "nc" not in _CACHE:
        _CACHE["nc"] = _build()
    return _CACHE["nc"]


def _prep_inputs(hidden_state, w_q, b_q, w_k, b_k, w_v, b_v, w_o, b_o):
    """Host-side sharding/layout prep. Only layout/dtype transforms."""
    xt = np.ascontiguousarray(hidden_state.transpose(0, 2, 1)).astype(NP_BF16)
    # wo pre-arranged to [P, 4, SC, 512] so each column-slice DMA line is
    # one contiguous 16KB run per partition
    wo = np.ascontiguousarray(
        w_o.reshape(SC, P, 4, 512).transpose(1, 2, 0, 3)).astype(NP_BF16)
    bo = np.broadcast_to(b_o.astype(np.float32), (P, HID)).copy()
    in_maps = []
    for g in range(NCORES):
        wq_g = np.ascontiguousarray(
            (w_q[:, g * QF:(g + 1) * QF] * 0.125)
            .reshape(HC, P, QF).transpose(1, 0, 2)).astype(NP_BF16)
        bq_g = np.ascontiguousarray(
            (b_q[g * QF:(g + 1) * QF] * 0.125).reshape(2, P)).astype(np.float32)
        wkv_g = np.ascontiguousarray(np.concatenate(
            [w_k[:, g * D:(g + 1) * D], w_v[:, g * D:(g + 1) * D]],
            axis=1).reshape(HC, P, P).transpose(1, 0, 2)).astype(NP_BF16)
        bkv_g = np.ascontiguousarray(np.concatenate(
            [b_k[g * D:(g + 1) * D], b_v[g * D:(g + 1) * D]])
            .reshape(P, 1)).astype(np.float32)
        in_maps.append({
            "xt": xt, "wq": wq_g, "bq": bq_g, "wkv": wkv_g, "bkv": bkv_g,
            "wo": wo, "bo": bo,
        })
    return in_maps


def kernel(hidden_state, w_q, b_q, w_k, b_k, w_v, b_v, w_o, b_o,
           _trace=False):
    hidden_state = np.asarray(hidden_state, np.float32)
    args = [np.asarray(a, np.float32) for a in
            (w_q, b_q, w_k, b_k, w_v, b_v, w_o, b_o)]
    nc = _get_nc()
    in_maps = _prep_inputs(hidden_state, *args)
    res = bass_utils.run_bass_kernel_spmd(
        nc, in_maps, core_ids=list(range(NCORES)), trace=_trace)
    out = np.concatenate([res.results[g]["out"] for g in range(NCORES)],
                         axis=1).astype(np.float32)
    if _trace:
        _CACHE["last_results"] = res
    return out
